# revision 13
# baseline (speedup 1.0000x reference)
"""DSA indexer kernel for Trainium2 (8 NeuronCores, sequence-parallel k).

scores[t, s] = causal_mask( sum_h w[t,h] * relu(q~[t,h] . k~[s]) * D^-0.5 )

where q~ = RoPE(q_latent @ Wq), k~ = RoPE(LN(x @ Wk)), w = x @ Wwt.
The reference also applies a Hadamard transform to both q~ and k~; it is
orthogonal and therefore preserved under the dot product, so it is skipped.

Sharding: query rows are stride-8 interleaved across the 8 cores so each
core runs an identical program (uniform causal block extents).  k-prep is
sequence-parallel: core c computes k~ only for its own 512-column s-chunk
(x @ Wk for that chunk) and the 8 chunks are exchanged with an in-kernel
AllGather (128 KB per rank, ~6-10us on TOPSP/SDMA silicon) that overlaps
with the q projection running on the tensor engine.

k-prep runs entirely in the transposed (d, s) layout:
  - LN mean-centering is folded into Wk on the host (linear in x).
  - LN variance = (1/D) sum_d kc^2 via a ones-column matmul, rstd broadcast
    back to 128 partitions via a rank-1 ones-row matmul.
  - RoPE uses the same permutation-matmul trick as the q side.
  - ln_g == 1 and ln_b == 0 are asserted host-side (true for this problem),
    so the affine part of LN is the identity.
The upper-triangular -1e9 region is constant and is filled on the host.
"""

import numpy as np
import ml_dtypes

import concourse.bass as bass
import concourse.mybir as mybir
import concourse.tile as tile
from concourse import bacc
from concourse.bass_utils import run_bass_kernel_spmd

F32 = mybir.dt.float32
BF16 = mybir.dt.bfloat16
NPBF16 = ml_dtypes.bfloat16

NCORES = 8
S = 4096          # sequence length
DMODEL = 2048
DCQ = 1536
H = 8
D = 128
R = 64
BASE = 10000.0
LN_EPS = 1e-5
NEG = -1e9
TPC = S // NCORES          # 512 t-rows per core
NTT = TPC // 128           # 4 t-tiles of 128 rows per core
KCH_X = DMODEL // 128      # 16 contraction chunks for x
KCH_Q = DCQ // 128         # 12 contraction chunks for q_latent
NSCH = S // 512            # 8 s-chunks of 512

AluOp = mybir.AluOpType
ActFn = mybir.ActivationFunctionType

# head-sum split per t-tile row: heads 0..NPE_J[j]-1 summed on PE via
# diag(w) matmuls, the rest on DVE scalar_tensor_tensor chains.  The last
# row runs while PE is draining, so it leans harder on PE.
NPE_J = (5, 5, 5, 6)
# relu split: heads in ACT_RELU drain their qk PSUM on the scalar engine,
# the rest on the vector engine.
ACT_RELU = (0, 1, 2, 3, 4, 5)

_CACHED = {}


def _build_program():
    nc = bacc.Bacc(
        "TRN2",
        target_bir_lowering=False,
        debug=False,
        enable_asserts=False,
        num_devices=NCORES,
    )

    def din(name, shape, dt):
        return nc.dram_tensor(name, list(shape), dt, kind="ExternalInput").ap()

    xk_d = din("xk", (128, KCH_X, 512), BF16)      # own s-chunk of xT
    xw_d = din("xw", (128, KCH_X, TPC), BF16)      # xT chunks, own t-row cols
    qlt_d = din("qlt", (128, KCH_Q, TPC), BF16)    # q_latentT chunks, own t cols
    wq_d = din("wq", (H, 128, KCH_Q, D), BF16)   # head-major for split loads
    wkc_d = din("wkc", (128, KCH_X, D), BF16)      # mean-centered Wk
    wwt_d = din("wwt", (128, KCH_X, H), BF16)      # includes D^-0.5
    takr_d = din("takr", (64, 512), BF16)          # k rope rows [cos|cos], own chunk
    tbkr_d = din("tbkr", (64, 512), BF16)          # k rope rows [-sin|sin], own chunk
    taq_d = din("taq", (64, TPC), BF16)            # q rope A rows (T layout)
    tbq_d = din("tbq", (64, TPC), BF16)            # q rope B rows
    perm_d = din("perm", (128, 128), BF16)         # rotate-half permutation
    idbf_d = din("idbf", (128, 128), BF16)         # identity (for diag weights)
    out_d = nc.dram_tensor("outd", [NTT, 128, S], BF16, kind="ExternalOutput").ap()

    with tile.TileContext(nc) as tc:
        with (
            tc.tile_pool(name="const", bufs=1) as cpool,
            tc.tile_pool(name="big", bufs=1) as xpool,
            tc.tile_pool(name="wqp", bufs=3) as wqpool,
            tc.tile_pool(name="work", bufs=4) as work,
            tc.tile_pool(name="stat", bufs=4) as stat,
            tc.tile_pool(name="acc", bufs=2) as accpool,
            tc.tile_pool(name="term", bufs=9) as termpool,
            tc.tile_pool(name="dwp", bufs=14) as dwpool,
            tc.tile_pool(name="psa", bufs=3, space="PSUM") as psa,
            tc.tile_pool(name="psb", bufs=2, space="PSUM") as psb,
            tc.tile_pool(name="dram", bufs=1, space="DRAM") as dram,
        ):
            # ---- k-prep path inputs: wkc first (gates every kprep matmul),
            # xk split across the sync and vector dma queues
            wkc_s = cpool.tile(list(wkc_d.shape), BF16, tag="wkc")
            nc.sync.dma_start(wkc_s[:], wkc_d)
            xk_s = xpool.tile([128, KCH_X, 512], BF16, tag="xk")
            nc.sync.dma_start(xk_s[:, 0:4, :], xk_d[:, 0:4, :])
            nc.gpsimd.dma_start(xk_s[:, 8:12, :], xk_d[:, 8:12, :])
            nc.sync.dma_start(xk_s[:, 4:8, :], xk_d[:, 4:8, :])
            nc.gpsimd.dma_start(xk_s[:, 12:16, :], xk_d[:, 12:16, :])
            xw_s = xpool.tile(list(xw_d.shape), BF16, tag="xw")
            nc.sync.dma_start(xw_s[:], xw_d)

            # ---- q path + small constants: scalar dma queue
            tkb = cpool.tile([128, 512], BF16, tag="tkb")
            nc.scalar.dma_start(tkb[0:64, :], takr_d)
            nc.scalar.dma_start(tkb[64:128, :], tbkr_d)
            takr_s = tkb[0:64, :]
            tbkr_s = tkb[64:128, :]
            perm_s = cpool.tile([128, 128], BF16, tag="perm")
            nc.scalar.dma_start(perm_s[:], perm_d)
            qlt_s = xpool.tile(list(qlt_d.shape), BF16, tag="qlt")
            for piece in range(3):
                csl = slice(piece * 4, (piece + 1) * 4)
                nc.scalar.dma_start(qlt_s[:, csl, :], qlt_d[:, csl, :])
            tqb = cpool.tile([128, TPC], BF16, tag="tqb")
            nc.scalar.dma_start(tqb[0:64, :], taq_d)
            nc.scalar.dma_start(tqb[64:128, :], tbq_d)
            taq_s = tqb[0:64, :]
            tbq_s = tqb[64:128, :]
            wq_h = []
            for h in range(H):
                t = wqpool.tile([128, KCH_Q, D], BF16, tag="wqh")
                nc.scalar.dma_start(t[:], wq_d[h])
                wq_h.append(t)
            idbf_s = cpool.tile([128, 128], BF16, tag="idbf")
            nc.scalar.dma_start(idbf_s[:], idbf_d)
            wwt_s = cpool.tile(list(wwt_d.shape), BF16, tag="wwt")
            nc.scalar.dma_start(wwt_s[:], wwt_d)

            kall = xpool.tile([128, NSCH, 512], BF16, tag="kall")  # full k~T
            qT = xpool.tile([128, H * TPC], BF16, tag="qT")  # final q~T
            w_all = xpool.tile([128, NTT, H], F32, tag="w_all")

            # AllGather bounce buffers (DRAM; collective runs on TOPSP/SDMA)
            kag_in = dram.tile([128, 512], BF16, tag="kag_in")
            kag_out = dram.tile([NCORES, 128, 512], BF16, tag="kag_out")

            # ---------- stage builders ----------
            onesr = cpool.tile([1, 128], BF16, tag="onesr")
            nc.vector.memset(onesr[:], 1.0)
            oneod = cpool.tile([128, 1], BF16, tag="oneod")
            nc.vector.memset(oneod[:], 1.0 / D)
            eps1 = cpool.tile([1, 1], F32, tag="eps1")
            nc.vector.memset(eps1[:], LN_EPS)

            def kprep_own():
                """Project + LN + rope own 512-col s-chunk, then AllGather."""
                ps = psa.tile([128, 1024], F32, tag="psa", name="kps")[:, :512]
                for cc in range(KCH_X):
                    nc.tensor.matmul(
                        ps,
                        lhsT=wkc_s[:, cc, :],
                        rhs=xk_s[:, cc, :],
                        start=(cc == 0),
                        stop=(cc == KCH_X - 1),
                    )
                # rstd of the mean-centered column, entirely at partition 0
                sq = work.tile([128, 512], BF16, tag="sq")
                nc.scalar.activation(sq[:], ps, ActFn.Square)
                vp = psb.tile([128, 512], F32, tag="psb", name="vps")[0:1, :]
                nc.tensor.matmul(vp, lhsT=oneod[:], rhs=sq[:], start=True, stop=True)
                sd = stat.tile([1, 512], F32, tag="sd")
                nc.scalar.activation(sd[:], vp, ActFn.Sqrt, bias=eps1[:])
                rsf = stat.tile([1, 512], F32, tag="rsf")
                nc.vector.reciprocal_approx_fast(rsf[:], sd[:])
                rsb = stat.tile([1, 512], BF16, tag="rsb")
                with nc.allow_low_precision(reason="rstd in bf16: 0.4% on k scale"):
                    nc.vector.tensor_copy(rsb[:], rsf[:])
                # drain kc, broadcast rstd (rank-1 matmul), apply
                kown = work.tile([128, 512], BF16, tag="kown")
                nc.scalar.copy(kown[:], ps)
                rb = psb.tile([128, 512], F32, tag="psb", name="rbps")
                nc.tensor.matmul(rb[:], lhsT=onesr[:], rhs=rsb[:], start=True, stop=True)
                nc.vector.tensor_tensor(kown[:], kown[:], rb[:], AluOp.mult)
                psr = psb.tile([128, 512], F32, tag="psb", name="krps")
                nc.tensor.matmul(psr[:], lhsT=perm_s[:], rhs=kown[:], start=True, stop=True)
                t1 = work.tile([64, 512], BF16, tag="t1")
                nc.vector.tensor_tensor(t1[:], kown[0:64, :], takr_s[:], AluOp.mult)
                t2 = work.tile([64, 512], BF16, tag="t2")
                nc.vector.tensor_tensor(t2[:], psr[0:64, :], tbkr_s[:], AluOp.mult)
                nc.vector.tensor_tensor(kown[0:64, :], t1[:], t2[:], AluOp.add)
                # exchange: own chunk out, all 8 chunks back
                nc.gpsimd.dma_start(kag_in[:], kown[:])
                nc.gpsimd.collective_compute(
                    "AllGather",
                    AluOp.bypass,
                    replica_groups=[list(range(NCORES))],
                    ins=[kag_in.opt()],
                    outs=[kag_out.opt()],
                )
                nc.gpsimd.dma_start(
                    kall[:, 0:4, :], kag_out[0:4, :, :].transpose([1, 0, 2])
                )
                nc.sync.dma_start(
                    kall[:, 4:8, :], kag_out[4:8, :, :].transpose([1, 0, 2])
                )

            def qproj_head(h):
                hsl = slice(h * TPC, (h + 1) * TPC)
                psq = psa.tile([128, 1024], F32, tag="psa", name="qps")[:, :512]
                for cc in range(KCH_Q):
                    nc.tensor.matmul(
                        psq,
                        lhsT=wq_h[h][:, cc, :],
                        rhs=qlt_s[:, cc, :],
                        start=(cc == 0),
                        stop=(cc == KCH_Q - 1),
                    )
                nc.scalar.copy(qT[:, hsl], psq)

            def qrope_head(h):
                hsl = slice(h * TPC, (h + 1) * TPC)
                psr = psb.tile([128, 512], F32, tag="psb", name="qrps")
                nc.tensor.matmul(psr[:], lhsT=perm_s[:], rhs=qT[:, hsl], start=True, stop=True)
                t1 = work.tile([64, 512], BF16, tag="t1")
                nc.vector.tensor_tensor(t1[:], qT[0:64, hsl], taq_s[:], AluOp.mult)
                t2 = work.tile([64, 512], BF16, tag="t2")
                nc.vector.tensor_tensor(t2[:], psr[0:64, :], tbq_s[:], AluOp.mult)
                nc.vector.tensor_tensor(qT[0:64, hsl], t1[:], t2[:], AluOp.add)

            def wproj_all():
                # w in (h, t) layout: 16 matmuls with 512-wide free dim
                # instead of 64 fill-dominated 8-wide ones
                psw = psb.tile([128, 512], F32, tag="psb", name="wps")[:H, :]
                for cc in range(KCH_X):
                    nc.tensor.matmul(
                        psw,
                        lhsT=wwt_s[:, cc, :],
                        rhs=xw_s[:, cc, :],
                        start=(cc == 0),
                        stop=(cc == KCH_X - 1),
                    )
                w8 = stat.tile([H, 512], BF16, tag="w8")
                nc.scalar.copy(w8[:], psw)
                return w8

            def wproj_tile(j, w8):
                # transpose the (h, 128-t) slab back to (t, h) on the PE
                pst = psb.tile([128, 1024], BF16, tag="psb", name="wtp")[:, :H]
                nc.tensor.transpose(
                    pst, w8[:, j * 128 : (j + 1) * 128], idbf_s[0:H, 0:H]
                )
                nc.vector.tensor_copy(w_all[:, j, :], pst)

            all_dws = {}

            def dw_tile(j, ndw):
                row = []
                for h in range(ndw):
                    dw = dwpool.tile([128, 128], BF16, tag="dw")
                    nc.vector.tensor_scalar_mul(dw[:], idbf_s[:], w_all[:, j, h : h + 1])
                    row.append(dw)
                all_dws[j] = row

            accs = {}

            def qk_block(j, db, act_all=False, npe=None, out_split=False):
                """t-tile j vs s-block [db*1024, (db+1)*1024)."""
                if db == 0:
                    accs[j] = accpool.tile([128, S], BF16, tag="acc", name="acc")
                acc = accs[j]
                terms = []
                for h in range(H):
                    lq = qT[:, h * TPC + j * 128 : h * TPC + (j + 1) * 128]
                    ps = psa.tile([128, 1024], F32, tag="psa", name="zps")
                    nc.tensor.matmul(
                        ps[:, 0:512], lhsT=lq, rhs=kall[:, 2 * db, :],
                        start=True, stop=True,
                    )
                    nc.tensor.matmul(
                        ps[:, 512:1024], lhsT=lq, rhs=kall[:, 2 * db + 1, :],
                        start=True, stop=True,
                    )
                    term = termpool.tile([128, 1024], BF16, tag="term")
                    if act_all or h in ACT_RELU:
                        nc.scalar.activation(term[:], ps[:], ActFn.Relu)
                    else:
                        nc.vector.tensor_scalar_max(term[:], ps[:], 0.0)
                    terms.append(term)
                if npe is None:
                    npe = NPE_J[j]
                for half in range(2):
                    csl = slice(half * 512, (half + 1) * 512)
                    sc = psb.tile([128, 512], F32, tag="psb", name="scps")
                    for i in range(npe):
                        nc.tensor.matmul(
                            sc[:], lhsT=all_dws[j][i][:], rhs=terms[i][:, csl],
                            start=(i == 0), stop=(i == npe - 1),
                        )
                    sl = acc[:, db * 1024 + half * 512 : db * 1024 + (half + 1) * 512]
                    if npe >= H:
                        nc.vector.tensor_copy(sl, sc[:])
                    else:
                        nc.vector.scalar_tensor_tensor(
                            sl, terms[npe][:, csl], w_all[:, j, npe : npe + 1], sc[:],
                            AluOp.mult, AluOp.add,
                        )
                        for i in range(npe + 1, H):
                            nc.vector.scalar_tensor_tensor(
                                sl, terms[i][:, csl], w_all[:, j, i : i + 1], sl,
                                AluOp.mult, AluOp.add,
                            )
                    if out_split:
                        nc.sync.dma_start(
                            out_d[j][:, db * 1024 + half * 512 : db * 1024 + (half + 1) * 512],
                            sl,
                        )
                if not out_split:
                    nc.gpsimd.dma_start(
                        out_d[j][:, db * 1024 : (db + 1) * 1024],
                        acc[:, db * 1024 : (db + 1) * 1024],
                    )

            # ---------- issue order: k-prep first (AllGather overlaps q side)
            kprep_own()
            qproj_head(0)
            qrope_head(0)
            qproj_head(1)
            qrope_head(1)
            qproj_head(2)
            qrope_head(2)
            qproj_head(3)
            qrope_head(3)
            qproj_head(4)
            qrope_head(4)
            qproj_head(5)
            qrope_head(5)
            qproj_head(6)
            qrope_head(6)
            qproj_head(7)
            qrope_head(7)
            w8 = wproj_all()
            wproj_tile(0, w8)
            dw_tile(0, NPE_J[0])
            wproj_tile(1, w8)
            dw_tile(1, NPE_J[1])
            qk_block(0, 0)
            qk_block(1, 0)
            qk_block(1, 1)
            wproj_tile(2, w8)
            dw_tile(2, NPE_J[2])
            qk_block(2, 0)
            qk_block(2, 1)
            qk_block(2, 2)
            wproj_tile(3, w8)
            dw_tile(3, H)
            qk_block(3, 0)
            qk_block(3, 1)
            qk_block(3, 2)
            qk_block(3, 3, act_all=True, npe=H, out_split=True)

    nc.compile()
    return nc


def _host_inputs(x, q_latent, Wq, Wk, ln_g, ln_b, Wwt):
    """Build the 8 per-core input dicts (all layout prep / constants)."""
    f32 = np.float32
    assert np.allclose(np.asarray(ln_g), 1.0), "kernel assumes ln_g == 1"
    assert np.allclose(np.asarray(ln_b), 0.0), "kernel assumes ln_b == 0"

    xT = np.ascontiguousarray(x[0].T.astype(f32))            # (2048, 4096)
    qlT = np.ascontiguousarray(q_latent[0].T.astype(f32))    # (1536, 4096)

    def chunk_T(a2d, kch, cols):
        # (K, n) col-slice -> (128, kch, n) contiguous
        sl = np.ascontiguousarray(a2d[:, cols])
        return np.ascontiguousarray(
            sl.reshape(kch, 128, sl.shape[1]).transpose(1, 0, 2)
        )

    xk_full = np.ascontiguousarray(
        xT.reshape(KCH_X, 128, NSCH, 512).transpose(2, 1, 0, 3)
    ).astype(NPBF16)

    wq_r = np.ascontiguousarray(
        np.asarray(Wq, f32).reshape(KCH_Q, 128, H, D).transpose(2, 1, 0, 3)
    ).astype(NPBF16)
    Wk_c = np.asarray(Wk, f32) - np.asarray(Wk, f32).mean(axis=1, keepdims=True)
    wkc_r = np.ascontiguousarray(
        Wk_c.reshape(KCH_X, 128, D).transpose(1, 0, 2)
    ).astype(NPBF16)
    wwt_r = np.ascontiguousarray(
        (np.asarray(Wwt, f32) * (D ** -0.5)).reshape(KCH_X, 128, H).transpose(1, 0, 2)
    ).astype(NPBF16)

    freqs = (BASE ** (-(np.arange(0, R, 2, dtype=f32) / R))).astype(f32)  # (32,)
    perm = np.zeros((128, 128), dtype=NPBF16)
    for m in range(32):
        perm[m + 32, m] = 1
        perm[m, m + 32] = 1
    idbf = np.eye(128, dtype=NPBF16)

    # k rope rows for all s
    ang_s = np.arange(S, dtype=f32)[:, None] * freqs          # (S, 32)
    cosT, sinT = np.cos(ang_s).T.astype(f32), np.sin(ang_s).T.astype(f32)
    takr = np.concatenate([cosT, cosT], axis=0).astype(NPBF16)   # (64, S)
    tbkr = np.concatenate([-sinT, sinT], axis=0).astype(NPBF16)

    in_maps = []
    for c in range(NCORES):
        tm = c + NCORES * np.arange(TPC)
        ssl = slice(c * 512, (c + 1) * 512)

        ang_q = tm[:, None].astype(f32) * freqs
        cosq, sinq = np.cos(ang_q).T.astype(f32), np.sin(ang_q).T.astype(f32)
        taq = np.ascontiguousarray(
            np.concatenate([cosq, cosq], axis=0)).astype(NPBF16)  # (64, 512)
        tbq = np.ascontiguousarray(
            np.concatenate([-sinq, sinq], axis=0)).astype(NPBF16)

        in_maps.append({
            "xk": xk_full[c],
            "xw": chunk_T(xT, KCH_X, tm).astype(NPBF16),
            "qlt": chunk_T(qlT, KCH_Q, tm).astype(NPBF16),
            "wq": wq_r,
            "wkc": wkc_r,
            "wwt": wwt_r,
            "takr": np.ascontiguousarray(takr[:, ssl]),
            "tbkr": np.ascontiguousarray(tbkr[:, ssl]),
            "taq": taq,
            "tbq": tbq,
            "perm": perm,
            "idbf": idbf,
        })
    return in_maps


def run(inputs, **spmd_kwargs):
    """Run on HW; returns (full scores (1,S,S) f32, BassKernelResults)."""
    if "nc" not in _CACHED:
        _CACHED["nc"] = _build_program()
    nc = _CACHED["nc"]
    in_maps = _host_inputs(**inputs)
    res = run_bass_kernel_spmd(nc, in_maps, core_ids=list(range(NCORES)), **spmd_kwargs)
    out = np.full((S, S), NEG, dtype=np.float32)
    for c in range(NCORES):
        tm = c + NCORES * np.arange(TPC)
        dev = res.results[c]["outd"].reshape(TPC, S).astype(np.float32)
        out[tm] = dev
    tri = np.triu_indices(S, k=1)
    out[tri] = NEG
    return out[None], res


def kernel(**inputs):
    out, _ = run(inputs)
    return out


if __name__ == "__main__":
    import sys
    if "--build" in sys.argv:
        _build_program()
        print("BUILD OK")


# revision 22
# speedup vs baseline: 1.0539x; 1.0539x over previous
"""DSA indexer kernel for Trainium2 (8 NeuronCores, sequence-parallel k).

scores[t, s] = causal_mask( sum_h w[t,h] * relu(q~[t,h] . k~[s]) * D^-0.5 )

where q~ = RoPE(q_latent @ Wq), k~ = RoPE(LN(x @ Wk)), w = x @ Wwt.
The reference also applies a Hadamard transform to both q~ and k~; it is
orthogonal and therefore preserved under the dot product, so it is skipped.

Sharding: query rows are stride-8 interleaved across the 8 cores so each
core runs an identical program (uniform causal block extents).  k-prep is
sequence-parallel: core c computes k~ only for its own 512-column s-chunk
(x @ Wk for that chunk) and the 8 chunks are exchanged with an in-kernel
AllGather (128 KB per rank, ~6-10us on TOPSP/SDMA silicon) that overlaps
with the q projection running on the tensor engine.

k-prep runs entirely in the transposed (d, s) layout:
  - LN mean-centering is folded into Wk on the host (linear in x).
  - LN variance = (1/D) sum_d kc^2 via a ones-column matmul, rstd broadcast
    back to 128 partitions via a rank-1 ones-row matmul.
  - RoPE uses the same permutation-matmul trick as the q side.
  - ln_g == 1 and ln_b == 0 are asserted host-side (true for this problem),
    so the affine part of LN is the identity.
The upper-triangular -1e9 region is constant and is filled on the host.
"""

import numpy as np
import ml_dtypes

import concourse.bass as bass
import concourse.mybir as mybir
import concourse.tile as tile
from concourse import bacc
from concourse.bass_utils import run_bass_kernel_spmd

F32 = mybir.dt.float32
BF16 = mybir.dt.bfloat16
NPBF16 = ml_dtypes.bfloat16

NCORES = 8
S = 4096          # sequence length
DMODEL = 2048
DCQ = 1536
H = 8
D = 128
R = 64
BASE = 10000.0
LN_EPS = 1e-5
NEG = -1e9
TPC = S // NCORES          # 512 t-rows per core
NTT = TPC // 128           # 4 t-tiles of 128 rows per core
KCH_X = DMODEL // 128      # 16 contraction chunks for x
KCH_Q = DCQ // 128         # 12 contraction chunks for q_latent
NSCH = S // 512            # 8 s-chunks of 512

AluOp = mybir.AluOpType
ActFn = mybir.ActivationFunctionType

# head-sum split per t-tile row: heads 0..NPE_J[j]-1 summed on PE via
# diag(w) matmuls, the rest on DVE scalar_tensor_tensor chains.  The last
# row runs while PE is draining, so it leans harder on PE.
NPE_J = (5, 5, 5, 6)
# relu split: heads in ACT_RELU drain their qk PSUM on the scalar engine,
# the rest on the vector engine.
ACT_RELU = (0, 1, 2, 3, 4, 5)

_CACHED = {}


def _build_program():
    nc = bacc.Bacc(
        "TRN2",
        target_bir_lowering=False,
        debug=False,
        enable_asserts=False,
        num_devices=NCORES,
    )

    def din(name, shape, dt):
        return nc.dram_tensor(name, list(shape), dt, kind="ExternalInput").ap()

    xk_d = din("xk", (128, KCH_X, 512), BF16)      # own s-chunk of xT
    xk01_d = din("xk01", (128, KCH_X, 1024), BF16)  # s-chunks 0,1 (local redundant)
    xw_d = din("xw", (128, KCH_X, TPC), BF16)      # xT chunks, own t-row cols
    qlt_d = din("qlt", (128, KCH_Q, TPC), BF16)    # q_latentT chunks, own t cols
    wq_d = din("wq", (H, 128, KCH_Q, D), BF16)   # head-major for split loads
    wkc_d = din("wkc", (128, KCH_X, D), BF16)      # mean-centered Wk
    wwt_d = din("wwt", (128, KCH_X, H), BF16)      # includes D^-0.5
    takr_d = din("takr", (64, 512), BF16)          # k rope rows [cos|cos], own chunk
    tbkr_d = din("tbkr", (64, 512), BF16)          # k rope rows [-sin|sin], own chunk
    takr01_d = din("takr01", (64, 1024), BF16)     # k rope rows, chunks 0-1
    tbkr01_d = din("tbkr01", (64, 1024), BF16)
    taq_d = din("taq", (64, TPC), BF16)            # q rope A rows (T layout)
    tbq_d = din("tbq", (64, TPC), BF16)            # q rope B rows
    perm_d = din("perm", (128, 128), BF16)         # rotate-half permutation
    idbf_d = din("idbf", (128, 128), BF16)         # identity (for diag weights)
    out_d = nc.dram_tensor("outd", [NTT, 128, S], BF16, kind="ExternalOutput").ap()

    with tile.TileContext(nc) as tc:
        with (
            tc.tile_pool(name="const", bufs=1) as cpool,
            tc.tile_pool(name="big", bufs=1) as xpool,
            tc.tile_pool(name="wqp", bufs=3) as wqpool,
            tc.tile_pool(name="work", bufs=4) as work,
            tc.tile_pool(name="stat", bufs=4) as stat,
            tc.tile_pool(name="acc", bufs=4) as accpool,
            tc.tile_pool(name="term", bufs=9) as termpool,
            tc.tile_pool(name="dwp", bufs=23) as dwpool,
            tc.tile_pool(name="psa", bufs=3, space="PSUM") as psa,
            tc.tile_pool(name="psb", bufs=2, space="PSUM") as psb,
            tc.tile_pool(name="dram", bufs=1, space="DRAM") as dram,
        ):
            # ---- k-prep path inputs: wkc first (gates every kprep matmul),
            # xk split across the sync and gpsimd dma queues
            wkc_s = cpool.tile(list(wkc_d.shape), BF16, tag="wkc")
            nc.sync.dma_start(wkc_s[:], wkc_d)
            xk_s = xpool.tile([128, KCH_X, 512], BF16, tag="xk")
            nc.sync.dma_start(xk_s[:, 0:4, :], xk_d[:, 0:4, :])
            nc.gpsimd.dma_start(xk_s[:, 8:12, :], xk_d[:, 8:12, :])
            nc.sync.dma_start(xk_s[:, 4:8, :], xk_d[:, 4:8, :])
            nc.gpsimd.dma_start(xk_s[:, 12:16, :], xk_d[:, 12:16, :])
            # redundant local copy of s-chunks 0,1: frees qk blocks db=0 from
            # any dependence on the AllGather (whose latency is ~36us+)
            xk01_s = xpool.tile([128, KCH_X, 1024], BF16, tag="xk01")
            nc.gpsimd.dma_start(xk01_s[:, :, 512:1024], xk01_d[:, :, 512:1024])
            nc.sync.dma_start(xk01_s[:, :, 0:512], xk01_d[:, :, 0:512])
            xw_s = xpool.tile(list(xw_d.shape), BF16, tag="xw")
            nc.sync.dma_start(xw_s[:], xw_d)

            # ---- q path + small constants: scalar dma queue
            tkb = cpool.tile([128, 512], BF16, tag="tkb")
            nc.scalar.dma_start(tkb[0:64, :], takr_d)
            nc.scalar.dma_start(tkb[64:128, :], tbkr_d)
            takr_s = tkb[0:64, :]
            tbkr_s = tkb[64:128, :]
            perm_s = cpool.tile([128, 128], BF16, tag="perm")
            nc.scalar.dma_start(perm_s[:], perm_d)
            tkb01 = cpool.tile([128, 1024], BF16, tag="tkb01")
            nc.scalar.dma_start(tkb01[0:64, :], takr01_d)
            nc.scalar.dma_start(tkb01[64:128, :], tbkr01_d)
            qlt_s = xpool.tile(list(qlt_d.shape), BF16, tag="qlt")
            for piece in range(3):
                csl = slice(piece * 4, (piece + 1) * 4)
                nc.scalar.dma_start(qlt_s[:, csl, :], qlt_d[:, csl, :])
            tqb = cpool.tile([128, TPC], BF16, tag="tqb")
            nc.scalar.dma_start(tqb[0:64, :], taq_d)
            nc.scalar.dma_start(tqb[64:128, :], tbq_d)
            taq_s = tqb[0:64, :]
            tbq_s = tqb[64:128, :]
            wq_h = []
            for h in range(H):
                t = wqpool.tile([128, KCH_Q, D], BF16, tag="wqh")
                nc.scalar.dma_start(t[:], wq_d[h])
                wq_h.append(t)
            idbf_s = cpool.tile([128, 128], BF16, tag="idbf")
            nc.scalar.dma_start(idbf_s[:], idbf_d)
            wwt_s = cpool.tile(list(wwt_d.shape), BF16, tag="wwt")
            nc.scalar.dma_start(wwt_s[:], wwt_d)

            kall = xpool.tile([128, NSCH, 512], BF16, tag="kall")  # full k~T
            qT = xpool.tile([128, H * TPC], BF16, tag="qT")  # final q~T
            w_all = xpool.tile([128, NTT, H], F32, tag="w_all")

            # AllGather bounce buffers (DRAM; collective runs on TOPSP/SDMA)
            kag_in = dram.tile([128, 512], BF16, tag="kag_in")
            kag_out = dram.tile([NCORES, 128, 512], BF16, tag="kag_out")

            # ---------- stage builders ----------
            onesr = cpool.tile([1, 128], BF16, tag="onesr")
            nc.vector.memset(onesr[:], 1.0)
            oneod = cpool.tile([128, 1], BF16, tag="oneod")
            nc.vector.memset(oneod[:], 1.0 / D)
            eps1 = cpool.tile([1, 1], F32, tag="eps1")
            nc.vector.memset(eps1[:], LN_EPS)

            def kprep_chunk(xt, csl, ta, tb, kout):
                """Project + LN + rope one 512-col s-chunk into kout (SBUF)."""
                ps = psa.tile([128, 1024], F32, tag="psa", name="kps")[:, :512]
                for cc in range(KCH_X):
                    nc.tensor.matmul(
                        ps,
                        lhsT=wkc_s[:, cc, :],
                        rhs=xt[:, cc, csl],
                        start=(cc == 0),
                        stop=(cc == KCH_X - 1),
                    )
                # rstd of the mean-centered column, entirely at partition 0
                sq = work.tile([128, 512], BF16, tag="sq")
                nc.scalar.activation(sq[:], ps, ActFn.Square)
                vp = psb.tile([128, 512], F32, tag="psb", name="vps")[0:1, :]
                nc.tensor.matmul(vp, lhsT=oneod[:], rhs=sq[:], start=True, stop=True)
                sd = stat.tile([1, 512], F32, tag="sd")
                nc.scalar.activation(sd[:], vp, ActFn.Sqrt, bias=eps1[:])
                rsf = stat.tile([1, 512], F32, tag="rsf")
                nc.vector.reciprocal_approx_fast(rsf[:], sd[:])
                rsb = stat.tile([1, 512], BF16, tag="rsb")
                with nc.allow_low_precision(reason="rstd in bf16: 0.4% on k scale"):
                    nc.vector.tensor_copy(rsb[:], rsf[:])
                # drain kc, broadcast rstd (rank-1 matmul), apply
                nc.scalar.copy(kout, ps)
                rb = psb.tile([128, 512], F32, tag="psb", name="rbps")
                nc.tensor.matmul(rb[:], lhsT=onesr[:], rhs=rsb[:], start=True, stop=True)
                nc.vector.tensor_tensor(kout, kout, rb[:], AluOp.mult)
                psr = psb.tile([128, 512], F32, tag="psb", name="krps")
                nc.tensor.matmul(psr[:], lhsT=perm_s[:], rhs=kout, start=True, stop=True)
                t1 = work.tile([64, 512], BF16, tag="t1")
                nc.vector.tensor_tensor(t1[:], kout[0:64], ta, AluOp.mult)
                t2 = work.tile([64, 512], BF16, tag="t2")
                nc.vector.tensor_tensor(t2[:], psr[0:64, :], tb, AluOp.mult)
                nc.vector.tensor_tensor(kout[0:64], t1[:], t2[:], AluOp.add)

            def kprep_own():
                """Own s-chunk -> AllGather; chunks 2-7 of kall come back."""
                kown = work.tile([128, 512], BF16, tag="kown")
                kprep_chunk(xk_s, slice(0, 512), takr_s[:], tbkr_s[:], kown[:])
                nc.gpsimd.dma_start(kag_in[:], kown[:])
                nc.gpsimd.collective_compute(
                    "AllGather",
                    AluOp.bypass,
                    replica_groups=[list(range(NCORES))],
                    ins=[kag_in.opt()],
                    outs=[kag_out.opt()],
                )
                nc.gpsimd.dma_start(
                    kall[:, 2:5, :], kag_out[2:5, :, :].transpose([1, 0, 2])
                )
                nc.sync.dma_start(
                    kall[:, 5:8, :], kag_out[5:8, :, :].transpose([1, 0, 2])
                )

            def kprep01():
                """Chunks 0,1 computed locally (identical on every core)."""
                for lc in (1, 0):
                    csl = slice(lc * 512, (lc + 1) * 512)
                    kprep_chunk(
                        xk01_s, csl,
                        tkb01[0:64, csl], tkb01[64:128, csl],
                        kall[:, lc, :],
                    )

            def qproj_head(h):
                hsl = slice(h * TPC, (h + 1) * TPC)
                psq = psa.tile([128, 1024], F32, tag="psa", name="qps")[:, :512]
                for cc in range(KCH_Q):
                    nc.tensor.matmul(
                        psq,
                        lhsT=wq_h[h][:, cc, :],
                        rhs=qlt_s[:, cc, :],
                        start=(cc == 0),
                        stop=(cc == KCH_Q - 1),
                    )
                nc.scalar.copy(qT[:, hsl], psq)

            def qrope_head(h):
                hsl = slice(h * TPC, (h + 1) * TPC)
                psr = psb.tile([128, 512], F32, tag="psb", name="qrps")
                nc.tensor.matmul(psr[:], lhsT=perm_s[:], rhs=qT[:, hsl], start=True, stop=True)
                t1 = work.tile([64, 512], BF16, tag="t1")
                nc.vector.tensor_tensor(t1[:], qT[0:64, hsl], taq_s[:], AluOp.mult)
                t2 = work.tile([64, 512], BF16, tag="t2")
                nc.vector.tensor_tensor(t2[:], psr[0:64, :], tbq_s[:], AluOp.mult)
                nc.vector.tensor_tensor(qT[0:64, hsl], t1[:], t2[:], AluOp.add)

            def wproj_all():
                # w in (h, t) layout: 16 matmuls with 512-wide free dim
                # instead of 64 fill-dominated 8-wide ones
                psw = psb.tile([128, 512], F32, tag="psb", name="wps")[:H, :]
                for cc in range(KCH_X):
                    nc.tensor.matmul(
                        psw,
                        lhsT=wwt_s[:, cc, :],
                        rhs=xw_s[:, cc, :],
                        start=(cc == 0),
                        stop=(cc == KCH_X - 1),
                    )
                w8 = stat.tile([H, 512], BF16, tag="w8")
                nc.scalar.copy(w8[:], psw)
                return w8

            def wproj_tile(j, w8):
                # transpose the (h, 128-t) slab back to (t, h) on the PE
                pst = psb.tile([128, 1024], BF16, tag="psb", name="wtp")[:, :H]
                nc.tensor.transpose(
                    pst, w8[:, j * 128 : (j + 1) * 128], idbf_s[0:H, 0:H]
                )
                nc.vector.tensor_copy(w_all[:, j, :], pst)

            all_dws = {}

            def dw_tile(j, ndw):
                row = []
                for h in range(ndw):
                    dw = dwpool.tile([128, 128], BF16, tag="dw")
                    nc.vector.tensor_scalar_mul(dw[:], idbf_s[:], w_all[:, j, h : h + 1])
                    row.append(dw)
                all_dws[j] = row

            accs = {}

            def qk_block(j, db, act_all=False, npe=None, out_split=False):
                """t-tile j vs s-block [db*1024, (db+1)*1024)."""
                if db == 0:
                    accs[j] = accpool.tile([128, S], BF16, tag="acc", name="acc")
                acc = accs[j]
                terms = []
                for h in range(H):
                    lq = qT[:, h * TPC + j * 128 : h * TPC + (j + 1) * 128]
                    ps = psa.tile([128, 1024], F32, tag="psa", name="zps")
                    nc.tensor.matmul(
                        ps[:, 0:512], lhsT=lq, rhs=kall[:, 2 * db, :],
                        start=True, stop=True,
                    )
                    nc.tensor.matmul(
                        ps[:, 512:1024], lhsT=lq, rhs=kall[:, 2 * db + 1, :],
                        start=True, stop=True,
                    )
                    term = termpool.tile([128, 1024], BF16, tag="term")
                    if act_all or h in ACT_RELU:
                        nc.scalar.activation(term[:], ps[:], ActFn.Relu)
                    else:
                        nc.vector.tensor_scalar_max(term[:], ps[:], 0.0)
                    terms.append(term)
                if npe is None:
                    npe = NPE_J[j]
                for half in range(2):
                    csl = slice(half * 512, (half + 1) * 512)
                    sc = psb.tile([128, 512], F32, tag="psb", name="scps")
                    for i in range(npe):
                        nc.tensor.matmul(
                            sc[:], lhsT=all_dws[j][i][:], rhs=terms[i][:, csl],
                            start=(i == 0), stop=(i == npe - 1),
                        )
                    sl = acc[:, db * 1024 + half * 512 : db * 1024 + (half + 1) * 512]
                    if npe >= H:
                        nc.vector.tensor_copy(sl, sc[:])
                    else:
                        nc.vector.scalar_tensor_tensor(
                            sl, terms[npe][:, csl], w_all[:, j, npe : npe + 1], sc[:],
                            AluOp.mult, AluOp.add,
                        )
                        for i in range(npe + 1, H):
                            nc.vector.scalar_tensor_tensor(
                                sl, terms[i][:, csl], w_all[:, j, i : i + 1], sl,
                                AluOp.mult, AluOp.add,
                            )
                    if out_split:
                        nc.sync.dma_start(
                            out_d[j][:, db * 1024 + half * 512 : db * 1024 + (half + 1) * 512],
                            sl,
                        )
                if not out_split:
                    nc.gpsimd.dma_start(
                        out_d[j][:, db * 1024 : (db + 1) * 1024],
                        acc[:, db * 1024 : (db + 1) * 1024],
                    )

            # ---------- issue order: own k chunk first (AllGather flies while
            # the q side and the local chunks 0,1 keep the PE busy); qk blocks
            # db-major so the first four need only the local chunks
            kprep_own()
            qproj_head(0)
            qrope_head(0)
            qproj_head(1)
            qrope_head(1)
            qproj_head(2)
            qrope_head(2)
            kprep01()
            qproj_head(3)
            qrope_head(3)
            qproj_head(4)
            qrope_head(4)
            qproj_head(5)
            qrope_head(5)
            qproj_head(6)
            qrope_head(6)
            qproj_head(7)
            qrope_head(7)
            w8 = wproj_all()
            wproj_tile(0, w8)
            dw_tile(0, NPE_J[0])
            wproj_tile(1, w8)
            dw_tile(1, NPE_J[1])
            wproj_tile(2, w8)
            dw_tile(2, NPE_J[2])
            wproj_tile(3, w8)
            dw_tile(3, H)
            qk_block(0, 0)
            qk_block(1, 0)
            qk_block(2, 0)
            qk_block(3, 0)
            qk_block(1, 1)
            qk_block(2, 1)
            qk_block(3, 1)
            qk_block(2, 2)
            qk_block(3, 2)
            qk_block(3, 3, act_all=True, npe=H, out_split=True)

    nc.compile()
    return nc


def _host_inputs(x, q_latent, Wq, Wk, ln_g, ln_b, Wwt):
    """Build the 8 per-core input dicts (all layout prep / constants)."""
    f32 = np.float32
    assert np.allclose(np.asarray(ln_g), 1.0), "kernel assumes ln_g == 1"
    assert np.allclose(np.asarray(ln_b), 0.0), "kernel assumes ln_b == 0"

    xT = np.ascontiguousarray(x[0].T.astype(f32))            # (2048, 4096)
    qlT = np.ascontiguousarray(q_latent[0].T.astype(f32))    # (1536, 4096)

    def chunk_T(a2d, kch, cols):
        # (K, n) col-slice -> (128, kch, n) contiguous
        sl = np.ascontiguousarray(a2d[:, cols])
        return np.ascontiguousarray(
            sl.reshape(kch, 128, sl.shape[1]).transpose(1, 0, 2)
        )

    xk_full = np.ascontiguousarray(
        xT.reshape(KCH_X, 128, NSCH, 512).transpose(2, 1, 0, 3)
    ).astype(NPBF16)
    xk01 = np.ascontiguousarray(
        np.concatenate([xk_full[0], xk_full[1]], axis=2)
    )

    wq_r = np.ascontiguousarray(
        np.asarray(Wq, f32).reshape(KCH_Q, 128, H, D).transpose(2, 1, 0, 3)
    ).astype(NPBF16)
    Wk_c = np.asarray(Wk, f32) - np.asarray(Wk, f32).mean(axis=1, keepdims=True)
    wkc_r = np.ascontiguousarray(
        Wk_c.reshape(KCH_X, 128, D).transpose(1, 0, 2)
    ).astype(NPBF16)
    wwt_r = np.ascontiguousarray(
        (np.asarray(Wwt, f32) * (D ** -0.5)).reshape(KCH_X, 128, H).transpose(1, 0, 2)
    ).astype(NPBF16)

    freqs = (BASE ** (-(np.arange(0, R, 2, dtype=f32) / R))).astype(f32)  # (32,)
    perm = np.zeros((128, 128), dtype=NPBF16)
    for m in range(32):
        perm[m + 32, m] = 1
        perm[m, m + 32] = 1
    idbf = np.eye(128, dtype=NPBF16)

    # k rope rows for all s
    ang_s = np.arange(S, dtype=f32)[:, None] * freqs          # (S, 32)
    cosT, sinT = np.cos(ang_s).T.astype(f32), np.sin(ang_s).T.astype(f32)
    takr = np.concatenate([cosT, cosT], axis=0).astype(NPBF16)   # (64, S)
    tbkr = np.concatenate([-sinT, sinT], axis=0).astype(NPBF16)

    in_maps = []
    for c in range(NCORES):
        tm = c + NCORES * np.arange(TPC)
        ssl = slice(c * 512, (c + 1) * 512)

        ang_q = tm[:, None].astype(f32) * freqs
        cosq, sinq = np.cos(ang_q).T.astype(f32), np.sin(ang_q).T.astype(f32)
        taq = np.ascontiguousarray(
            np.concatenate([cosq, cosq], axis=0)).astype(NPBF16)  # (64, 512)
        tbq = np.ascontiguousarray(
            np.concatenate([-sinq, sinq], axis=0)).astype(NPBF16)

        in_maps.append({
            "xk": xk_full[c],
            "xk01": xk01,
            "xw": chunk_T(xT, KCH_X, tm).astype(NPBF16),
            "qlt": chunk_T(qlT, KCH_Q, tm).astype(NPBF16),
            "wq": wq_r,
            "wkc": wkc_r,
            "wwt": wwt_r,
            "takr": np.ascontiguousarray(takr[:, ssl]),
            "tbkr": np.ascontiguousarray(tbkr[:, ssl]),
            "takr01": np.ascontiguousarray(takr[:, 0:1024]),
            "tbkr01": np.ascontiguousarray(tbkr[:, 0:1024]),
            "taq": taq,
            "tbq": tbq,
            "perm": perm,
            "idbf": idbf,
        })
    return in_maps


def run(inputs, **spmd_kwargs):
    """Run on HW; returns (full scores (1,S,S) f32, BassKernelResults)."""
    if "nc" not in _CACHED:
        _CACHED["nc"] = _build_program()
    nc = _CACHED["nc"]
    in_maps = _host_inputs(**inputs)
    res = run_bass_kernel_spmd(nc, in_maps, core_ids=list(range(NCORES)), **spmd_kwargs)
    out = np.full((S, S), NEG, dtype=np.float32)
    for c in range(NCORES):
        tm = c + NCORES * np.arange(TPC)
        dev = res.results[c]["outd"].reshape(TPC, S).astype(np.float32)
        out[tm] = dev
    tri = np.triu_indices(S, k=1)
    out[tri] = NEG
    return out[None], res


def kernel(**inputs):
    out, _ = run(inputs)
    return out


if __name__ == "__main__":
    import sys
    if "--build" in sys.argv:
        _build_program()
        print("BUILD OK")
